# revision 11
# baseline (speedup 1.0000x reference)
"""EpsGINConv TRN2 kernel: 1-byte edge streams, DVE pair-summed u8 codes +
fp8 DoubleRow deep edges, identity-stationary transpose-accumulate.

Host pre-gathers x[src] per edge into a dst-window slot layout (window =
128 destinations on one core's partition lanes; 4-window groups share a
512-col PSUM bank). Every edge feature ships in ONE byte:

  - The first SPLIT edges per destination plus a folded self-edge
    (carrying (1+eps)*x[dst]) ship as uint8 codes round(v/s)+128 with
    s = 2^-5. The DVE sums code blocks PAIRWISE during upconversion
    (tensor_add u8+u8 -> fp16; code sums <= 2048 stay fp16-exact), which
    halves the PE matmul count. PSUM accumulates raw code sums; the
    affine (codes - 128*U_g)*s is folded into a pre-scaled fp16
    W1' = s*W1 and a per-group bias b1 - 128*s*U_g*colsum(W1).
  - Deeper edges ship as fp8-e4m3(x/s) (exact power-of-2 scale) and
    accumulate two-blocks-per-matmul via DoubleRow with an [I|I]
    identity.

All aggregation matmuls keep the identity STATIONARY, so blocks stream
at the N/2.4GHz column rate with no per-block weight-reload stall.
h = PSUM directly (pads quantize to exact code 128 / fp8 zero), then the
2-layer ReLU MLP with N=512 moving operands. Per-core HBM ~12.5MB.
"""
import sys

import numpy as np

if "/opt/trn_rl_repo" not in sys.path:
    sys.path.insert(0, "/opt/trn_rl_repo")

import ml_dtypes
import concourse.bass as bass
import concourse.bacc as bacc
import concourse.tile as tile
import concourse.mybir as mybir
from concourse.bass_utils import run_bass_kernel_spmd

P = 128
N_NODES = 50000
D = 128
N_CORES = 8
GW = 4
NW = 49
NPAD = NW * P  # 6272
GROUPS = [(g * GW, GW) for g in range(12)] + [(48, 1)]
PROC_ORDER = [12] + list(range(12))  # tiny group first for fast warmup
SPLIT = 7
S = 4.0 / 128.0  # 2^-5
CHF_P = 4  # fp8 chunk size in DoubleRow pairs

F32 = mybir.dt.float32
BF16 = mybir.dt.bfloat16
FP16 = mybir.dt.float16
FP8 = mybir.dt.float8e4
U8 = mybir.dt.uint8
Relu = mybir.ActivationFunctionType.Relu
Copy = mybir.ActivationFunctionType.Copy
DR = mybir.MatmulPerfMode.DoubleRow
BF = ml_dtypes.bfloat16
F8 = ml_dtypes.float8_e4m3


def _prep_host(edge_index):
    src = np.asarray(edge_index[0], dtype=np.int64)
    dst = np.asarray(edge_index[1], dtype=np.int64)

    deg_all = np.bincount(dst, minlength=N_NODES)
    gorder = np.argsort(-deg_all, kind="stable")
    core_of = np.empty(N_NODES, dtype=np.int64)
    crank_of = np.empty(N_NODES, dtype=np.int64)
    grank = np.arange(N_NODES)
    core_of[gorder] = grank % N_CORES
    crank_of[gorder] = grank // N_CORES
    node_at = np.full((N_CORES, NPAD), -1, dtype=np.int64)
    node_at[core_of[gorder], crank_of[gorder]] = gorder

    e_core = core_of[dst]
    e_rank = crank_of[dst]

    per_core = []
    wmax = np.zeros((N_CORES, NW), dtype=np.int64)
    for c in range(N_CORES):
        m = e_core == c
        r = e_rank[m]
        s_ = src[m]
        o = np.argsort(r, kind="stable")
        r = r[o]
        s_ = s_[o]
        cr = np.bincount(r, minlength=NPAD)
        wmax[c] = cr.reshape(NW, P).max(axis=1)
        per_core.append((r, s_, cr))

    B = wmax.max(axis=0)

    U_g, P_g = [], []
    for w0, gw in GROUPS:
        Bw = B[w0 : w0 + gw]
        U_g.append(int(np.minimum(Bw, SPLIT).max()) + 1)
        P_g.append(int(-(-int(np.maximum(Bw - SPLIT, 0).max()) // 2)))
    OU = np.concatenate([[0], np.cumsum([u * gw * P for u, (_, gw) in zip(U_g, GROUPS)])])
    OF = np.concatenate([[0], np.cumsum([p * 2 * gw * P for p, (_, gw) in zip(P_g, GROUPS)])])
    SU = int(OU[-1])
    SF = max(int(OF[-1]), P)

    PADROW = 2 * N_NODES
    idx_u8 = np.full((N_CORES, SU), PADROW, dtype=np.int64)
    idx_f8 = np.full((N_CORES, SF), PADROW, dtype=np.int64)

    g_of_w = np.minimum(np.arange(NW) // GW, len(GROUPS) - 1)
    w0_of_g = np.array([w0 for w0, _ in GROUPS])
    gw_of_g = np.array([gw for _, gw in GROUPS])
    OU_of_g = OU[:-1]
    OF_of_g = OF[:-1]
    Ug_arr = np.array(U_g, dtype=np.int64)
    npair_of_g = Ug_arr // 2

    def quad_of(gs, bjs):
        # stream quad order: [pair a-members | pair b-members | odd single]
        npair = npair_of_g[gs]
        pairpart = np.where(bjs % 2 == 0, bjs // 2, npair + bjs // 2)
        return np.where(bjs < 2 * npair, pairpart, 2 * npair)

    for c in range(N_CORES):
        r, s_, cr = per_core[c]
        firsts = np.concatenate([[0], np.cumsum(cr)])[:-1]
        j = np.arange(len(r)) - firsts[r]
        w = r // P
        lane = r % P
        g = g_of_w[w]
        wi = w - w0_of_g[g]
        gwg = gw_of_g[g]
        lo = j < SPLIT
        bj = j[lo] + 1  # block 0 is the self-edge
        quad = quad_of(g[lo], bj)
        colu = OU_of_g[g[lo]] + (quad * gwg[lo] + wi[lo]) * P + lane[lo]
        idx_u8[c, colu] = s_[lo]
        hi = ~lo
        dj = j[hi] - SPLIT
        q = dj // 2
        k = dj % 2
        colf = OF_of_g[g[hi]] + ((q * 2 + k) * gwg[hi] + wi[hi]) * P + lane[hi]
        idx_f8[c, colf] = s_[hi]
        # self-edges (block 0)
        rank_all = np.arange(NPAD)
        wS = rank_all // P
        laneS = rank_all % P
        gS = g_of_w[wS]
        wiS = wS - w0_of_g[gS]
        quadS = quad_of(gS, np.zeros(NPAD, dtype=np.int64))
        colS = OU_of_g[gS] + (quadS * gw_of_g[gS] + wiS) * P + laneS
        ids = node_at[c]
        valid = ids >= 0
        idx_u8[c, colS[valid]] = N_NODES + ids[valid]

    geo = dict(U_g=U_g, P_g=P_g, OU=OU_of_g, OF=OF_of_g, SU=SU, SF=SF)
    return node_at, geo, idx_u8, idx_f8


def _build_program(geo):
    nc = bacc.Bacc("TRN2", target_bir_lowering=False, debug=False, num_devices=N_CORES)
    xu8_d = nc.dram_tensor("xu8", [P, geo["SU"]], U8, kind="ExternalInput").ap()
    xf8_d = nc.dram_tensor("xf8", [P, geo["SF"]], FP8, kind="ExternalInput").ap()
    i16_d = nc.dram_tensor("identf", [P, P], FP16, kind="ExternalInput").ap()
    i8dr_d = nc.dram_tensor("ident8dr", [P, 2, P], FP8, kind="ExternalInput").ap()
    w1_d = nc.dram_tensor("w1s", [D, D], FP16, kind="ExternalInput").ap()
    w2_d = nc.dram_tensor("w2", [D, D], BF16, kind="ExternalInput").ap()
    b1_d = nc.dram_tensor("b1g", [P, len(GROUPS)], F32, kind="ExternalInput").ap()
    b2_d = nc.dram_tensor("b2c", [P, 1], F32, kind="ExternalInput").ap()
    outT_d = nc.dram_tensor("outT", [P, NPAD], BF16, kind="ExternalOutput").ap()

    U_g, P_g = geo["U_g"], geo["P_g"]
    OU, OF = geo["OU"], geo["OF"]

    with tile.TileContext(nc) as tc:
        with (
            tc.tile_pool(name="const", bufs=1) as cp,
            tc.tile_pool(name="u8ch", bufs=6) as u8p,
            tc.tile_pool(name="dq", bufs=6) as dqp,
            tc.tile_pool(name="f8ch", bufs=5) as f8p,
            tc.tile_pool(name="hbuf", bufs=3) as hp,
            tc.tile_pool(name="zbuf", bufs=3) as zp,
            tc.tile_pool(name="obuf", bufs=3) as op_,
            tc.tile_pool(name="ph", bufs=4, space="PSUM") as php,
            tc.tile_pool(name="pz", bufs=2, space="PSUM") as pzp,
            tc.tile_pool(name="po", bufs=2, space="PSUM") as pop,
        ):
            i16_t = cp.tile([P, P], FP16)
            nc.scalar.dma_start(i16_t[:], i16_d[:])
            i8dr_t = cp.tile([P, 2, P], FP8)
            nc.scalar.dma_start(i8dr_t[:], i8dr_d[:])
            w1_t = cp.tile([D, D], FP16)
            nc.scalar.dma_start(w1_t[:], w1_d[:])
            w2_t = cp.tile([D, D], BF16)
            nc.scalar.dma_start(w2_t[:], w2_d[:])
            b1_t = cp.tile([P, len(GROUPS)], F32)
            nc.scalar.dma_start(b1_t[:], b1_d[:])
            b2_t = cp.tile([P, 1], F32)
            nc.scalar.dma_start(b2_t[:], b2_d[:])

            PREFETCH = 3

            def issue_dmas(gi):
                w0, gw = GROUPS[gi]
                gcols = gw * P
                ug, pg = U_g[gi], P_g[gi]
                u8t = u8p.tile([P, ug, gcols], U8, tag=f"u8_{ug}_{gw}")
                nc.sync.dma_start(u8t[:], xu8_d[:, OU[gi] : OU[gi] + ug * gcols])
                f8_tiles = []
                q0 = 0
                while q0 < pg:
                    nq = min(CHF_P, pg - q0)
                    t = f8p.tile([P, nq, 2, gcols], FP8, tag=f"f8_{nq}_{gw}")
                    nc.gpsimd.dma_start(
                        t[:],
                        xf8_d[:, OF[gi] + q0 * 2 * gcols : OF[gi] + (q0 + nq) * 2 * gcols],
                    )
                    f8_tiles.append((t, q0, nq))
                    q0 += nq
                return u8t, f8_tiles

            staged = {}
            for idx in range(min(PREFETCH, len(PROC_ORDER))):
                staged[PROC_ORDER[idx]] = issue_dmas(PROC_ORDER[idx])

            def emit_w1(gk, h_sb):
                w0k, gwk = GROUPS[gk]
                gc = gwk * P
                psum_z = pzp.tile([P, GW * P], F32, tag="pz")
                nc.tensor.matmul(
                    psum_z[:, :gc], lhsT=w1_t[:], rhs=h_sb[:], start=True, stop=True
                )
                z_sb = zp.tile([P, gc], BF16, tag=f"z{gwk}")
                nc.scalar.activation(
                    z_sb[:], psum_z[:, :gc], Relu, bias=b1_t[:, gk : gk + 1]
                )
                return (gk, z_sb)

            def emit_w2(gj, z_sb):
                w0j, gwj = GROUPS[gj]
                gc = gwj * P
                psum_o = pop.tile([P, GW * P], F32, tag="po")
                nc.tensor.matmul(
                    psum_o[:, :gc], lhsT=w2_t[:], rhs=z_sb[:], start=True, stop=True
                )
                o_sb = op_.tile([P, gc], BF16, tag=f"o{gwj}")
                nc.scalar.activation(
                    o_sb[:], psum_o[:, :gc], Relu, bias=b2_t[:, :1]
                )
                nc.sync.dma_start(outT_d[:, w0j * P : (w0j + gwj) * P], o_sb[:])

            pend1 = []  # (gi, h_sb) awaiting the W1 matmul
            pend2 = []  # (gi, z_sb) awaiting the W2 matmul
            psum_of = {}
            dr_done = set()

            def emit_dr(gi):
                # fp8 DR accumulation for group gi (only needs prefetched DMA,
                # not the DVE chain) - runs a group ahead to fill PE ramp
                w0i, gwi = GROUPS[gi]
                gc = gwi * P
                pgi = P_g[gi]
                psum_h = php.tile([P, GW * P], F32, tag="ph")
                psum_of[gi] = psum_h
                u8ti, f8_tiles_i = staged[gi]
                for t, q0, nq in f8_tiles_i:
                    for qq in range(nq):
                        q = q0 + qq
                        nc.tensor.matmul(
                            psum_h[:, :gc],
                            lhsT=i8dr_t[:],
                            rhs=t[:, qq],
                            start=(q == 0),
                            stop=False,
                            perf_mode=DR,
                            skip_group_check=True,
                        )
                dr_done.add(gi)

            for idx, gi in enumerate(PROC_ORDER):
                w0, gw = GROUPS[gi]
                gcols = gw * P
                ug, pg = U_g[gi], P_g[gi]
                npair = ug // 2
                nsing = ug % 2
                nslot = npair + nsing
                if gi not in dr_done:
                    emit_dr(gi)
                u8t, f8_tiles = staged.pop(gi)

                dq = dqp.tile([P, nslot, gcols], FP16, tag=f"dq_{nslot}_{gw}")
                hh = (npair + 1) // 2
                for p0, p1 in ((0, hh), (hh, npair)):
                    if p1 > p0:
                        nc.vector.tensor_add(
                            dq[:, p0:p1],
                            u8t[:, p0:p1],
                            u8t[:, npair + p0 : npair + p1],
                        )
                if nsing:
                    nc.vector.tensor_copy(dq[:, npair], u8t[:, 2 * npair])

                # DR for the NEXT group ahead of this group's u8 matmuls
                if idx + 1 < len(PROC_ORDER) and PROC_ORDER[idx + 1] in staged:
                    emit_dr(PROC_ORDER[idx + 1])

                psum_h = psum_of.pop(gi)
                for slot in range(nslot):
                    nc.tensor.matmul(
                        psum_h[:, :gcols],
                        lhsT=i16_t[:],
                        rhs=dq[:, slot],
                        start=(slot == 0 and pg == 0),
                        stop=(slot == nslot - 1),
                        skip_group_check=True,
                    )

                h_sb = hp.tile([P, gw * P], FP16, tag=f"h{gw}")
                nc.scalar.activation(h_sb[:], psum_h[:, :gcols], Copy)

                if pend2:
                    emit_w2(*pend2.pop(0))
                if pend1:
                    pend2.append(emit_w1(*pend1.pop(0)))
                pend1.append((gi, h_sb))

                if idx + PREFETCH < len(PROC_ORDER):
                    gnext = PROC_ORDER[idx + PREFETCH]
                    staged[gnext] = issue_dmas(gnext)

            while pend1 or pend2:
                if pend2:
                    emit_w2(*pend2.pop(0))
                if pend1:
                    pend2.append(emit_w1(*pend1.pop(0)))
    nc.compile()
    return nc


def kernel(x, edge_index, W1, b1, W2, b2, eps):
    x = np.ascontiguousarray(np.asarray(x, dtype=np.float32))
    W1 = np.asarray(W1, dtype=np.float32)
    W2 = np.asarray(W2, dtype=np.float32)
    b1 = np.asarray(b1, dtype=np.float32)
    b2 = np.asarray(b2, dtype=np.float32)
    eps_val = float(np.asarray(eps))

    node_at, geo, idx_u8, idx_f8 = _prep_host(np.asarray(edge_index))
    nc = _build_program(geo)

    codes_all = np.empty((2 * N_NODES + 1, D), dtype=np.uint8)
    codes_all[:N_NODES] = (np.clip(np.rint(x / S), -128, 127) + 128).astype(np.uint8)
    codes_all[N_NODES : 2 * N_NODES] = (
        np.clip(np.rint((1.0 + eps_val) * x / S), -128, 127) + 128
    ).astype(np.uint8)
    codes_all[2 * N_NODES] = 128

    f8_all = np.zeros((2 * N_NODES + 1, D), dtype=F8)
    f8_all[:N_NODES] = (x / S).astype(F8)

    identf = np.eye(P, dtype=np.float32).astype(np.float16)
    i8dr = np.ascontiguousarray(
        np.stack([np.eye(P, dtype=np.float32).astype(F8)] * 2, axis=1)
    )
    w1s = (S * W1).astype(np.float16)
    colsum_w1 = W1.sum(axis=0)
    b1g = np.empty((P, len(GROUPS)), dtype=np.float32)
    for gi in range(len(GROUPS)):
        b1g[:, gi] = b1 - 128.0 * S * geo["U_g"][gi] * colsum_w1
    b2c = np.ascontiguousarray(b2.reshape(P, 1))

    in_maps = []
    for c in range(N_CORES):
        xu8 = np.ascontiguousarray(codes_all[idx_u8[c]].T)
        xf8 = np.ascontiguousarray(f8_all[idx_f8[c]].T)
        in_maps.append(
            {
                "xu8": xu8,
                "xf8": xf8,
                "identf": identf,
                "ident8dr": i8dr,
                "w1s": w1s,
                "w2": W2.astype(BF),
                "b1g": b1g,
                "b2c": b2c,
            }
        )
    res = run_bass_kernel_spmd(nc, in_maps, list(range(N_CORES)))

    out = np.empty((N_NODES, D), dtype=np.float32)
    for c in range(N_CORES):
        rows = np.asarray(res.results[c]["outT"]).astype(np.float32).T
        ids = node_at[c]
        valid = ids >= 0
        out[ids[valid]] = rows[valid]
    kernel.last_results = res
    return out


# revision 13
# speedup vs baseline: 1.0355x; 1.0355x over previous
"""EpsGINConv TRN2 kernel: 1-byte edge streams, DVE pair-summed u8 codes +
fp8 DoubleRow deep edges, identity-stationary transpose-accumulate.

Host pre-gathers x[src] per edge into a dst-window slot layout (window =
128 destinations on one core's partition lanes; 4-window groups share a
512-col PSUM bank). Every edge feature ships in ONE byte:

  - The first SPLIT edges per destination plus a folded self-edge
    (carrying (1+eps)*x[dst]) ship as uint8 codes round(v/s)+128 with
    s = 2^-5. The DVE sums code blocks PAIRWISE during upconversion
    (tensor_add u8+u8 -> fp16; code sums <= 2048 stay fp16-exact), which
    halves the PE matmul count. PSUM accumulates raw code sums; the
    affine (codes - 128*U_g)*s is folded into a pre-scaled fp16
    W1' = s*W1 and a per-group bias b1 - 128*s*U_g*colsum(W1).
  - Deeper edges ship as fp8-e4m3(x/s) (exact power-of-2 scale) and
    accumulate two-blocks-per-matmul via DoubleRow with an [I|I]
    identity.

All aggregation matmuls keep the identity STATIONARY, so blocks stream
at the N/2.4GHz column rate with no per-block weight-reload stall.
h = PSUM directly (pads quantize to exact code 128 / fp8 zero), then the
2-layer ReLU MLP with N=512 moving operands. Per-core HBM ~12.5MB.
"""
import sys

import numpy as np

if "/opt/trn_rl_repo" not in sys.path:
    sys.path.insert(0, "/opt/trn_rl_repo")

import ml_dtypes
import concourse.bass as bass
import concourse.bacc as bacc
import concourse.tile as tile
import concourse.mybir as mybir
from concourse.bass_utils import run_bass_kernel_spmd

P = 128
N_NODES = 50000
D = 128
N_CORES = 8
GW = 4
NW = 49
NPAD = NW * P  # 6272
GROUPS = [(g * GW, GW) for g in range(12)] + [(48, 1)]
PROC_ORDER = list(range(12, -1, -1))  # ascending data size: ramp is DMA-BW-bound
SPLIT = 7
S = 4.0 / 128.0  # 2^-5
CHF_P = 4  # fp8 chunk size in DoubleRow pairs

F32 = mybir.dt.float32
BF16 = mybir.dt.bfloat16
FP16 = mybir.dt.float16
FP8 = mybir.dt.float8e4
U8 = mybir.dt.uint8
Relu = mybir.ActivationFunctionType.Relu
Copy = mybir.ActivationFunctionType.Copy
DR = mybir.MatmulPerfMode.DoubleRow
BF = ml_dtypes.bfloat16
F8 = ml_dtypes.float8_e4m3


def _prep_host(edge_index):
    src = np.asarray(edge_index[0], dtype=np.int64)
    dst = np.asarray(edge_index[1], dtype=np.int64)

    deg_all = np.bincount(dst, minlength=N_NODES)
    gorder = np.argsort(-deg_all, kind="stable")
    core_of = np.empty(N_NODES, dtype=np.int64)
    crank_of = np.empty(N_NODES, dtype=np.int64)
    grank = np.arange(N_NODES)
    core_of[gorder] = grank % N_CORES
    crank_of[gorder] = grank // N_CORES
    node_at = np.full((N_CORES, NPAD), -1, dtype=np.int64)
    node_at[core_of[gorder], crank_of[gorder]] = gorder

    e_core = core_of[dst]
    e_rank = crank_of[dst]

    per_core = []
    wmax = np.zeros((N_CORES, NW), dtype=np.int64)
    for c in range(N_CORES):
        m = e_core == c
        r = e_rank[m]
        s_ = src[m]
        o = np.argsort(r, kind="stable")
        r = r[o]
        s_ = s_[o]
        cr = np.bincount(r, minlength=NPAD)
        wmax[c] = cr.reshape(NW, P).max(axis=1)
        per_core.append((r, s_, cr))

    B = wmax.max(axis=0)

    U_g, P_g = [], []
    for w0, gw in GROUPS:
        Bw = B[w0 : w0 + gw]
        U_g.append(int(np.minimum(Bw, SPLIT).max()) + 1)
        P_g.append(int(-(-int(np.maximum(Bw - SPLIT, 0).max()) // 2)))
    OU = np.concatenate([[0], np.cumsum([u * gw * P for u, (_, gw) in zip(U_g, GROUPS)])])
    OF = np.concatenate([[0], np.cumsum([p * 2 * gw * P for p, (_, gw) in zip(P_g, GROUPS)])])
    SU = int(OU[-1])
    SF = max(int(OF[-1]), P)

    PADROW = 2 * N_NODES
    idx_u8 = np.full((N_CORES, SU), PADROW, dtype=np.int64)
    idx_f8 = np.full((N_CORES, SF), PADROW, dtype=np.int64)

    g_of_w = np.minimum(np.arange(NW) // GW, len(GROUPS) - 1)
    w0_of_g = np.array([w0 for w0, _ in GROUPS])
    gw_of_g = np.array([gw for _, gw in GROUPS])
    OU_of_g = OU[:-1]
    OF_of_g = OF[:-1]
    Ug_arr = np.array(U_g, dtype=np.int64)
    npair_of_g = Ug_arr // 2

    def quad_of(gs, bjs):
        # stream quad order: [pair a-members | pair b-members | odd single]
        npair = npair_of_g[gs]
        pairpart = np.where(bjs % 2 == 0, bjs // 2, npair + bjs // 2)
        return np.where(bjs < 2 * npair, pairpart, 2 * npair)

    for c in range(N_CORES):
        r, s_, cr = per_core[c]
        firsts = np.concatenate([[0], np.cumsum(cr)])[:-1]
        j = np.arange(len(r)) - firsts[r]
        w = r // P
        lane = r % P
        g = g_of_w[w]
        wi = w - w0_of_g[g]
        gwg = gw_of_g[g]
        lo = j < SPLIT
        bj = j[lo] + 1  # block 0 is the self-edge
        quad = quad_of(g[lo], bj)
        colu = OU_of_g[g[lo]] + (quad * gwg[lo] + wi[lo]) * P + lane[lo]
        idx_u8[c, colu] = s_[lo]
        hi = ~lo
        dj = j[hi] - SPLIT
        q = dj // 2
        k = dj % 2
        colf = OF_of_g[g[hi]] + ((q * 2 + k) * gwg[hi] + wi[hi]) * P + lane[hi]
        idx_f8[c, colf] = s_[hi]
        # self-edges (block 0)
        rank_all = np.arange(NPAD)
        wS = rank_all // P
        laneS = rank_all % P
        gS = g_of_w[wS]
        wiS = wS - w0_of_g[gS]
        quadS = quad_of(gS, np.zeros(NPAD, dtype=np.int64))
        colS = OU_of_g[gS] + (quadS * gw_of_g[gS] + wiS) * P + laneS
        ids = node_at[c]
        valid = ids >= 0
        idx_u8[c, colS[valid]] = N_NODES + ids[valid]

    geo = dict(U_g=U_g, P_g=P_g, OU=OU_of_g, OF=OF_of_g, SU=SU, SF=SF)
    return node_at, geo, idx_u8, idx_f8


def _build_program(geo):
    nc = bacc.Bacc("TRN2", target_bir_lowering=False, debug=False, num_devices=N_CORES)
    xu8_d = nc.dram_tensor("xu8", [P, geo["SU"]], U8, kind="ExternalInput").ap()
    xf8_d = nc.dram_tensor("xf8", [P, geo["SF"]], FP8, kind="ExternalInput").ap()
    i16_d = nc.dram_tensor("identf", [P, P], FP16, kind="ExternalInput").ap()
    i8dr_d = nc.dram_tensor("ident8dr", [P, 2, P], FP8, kind="ExternalInput").ap()
    w1_d = nc.dram_tensor("w1s", [D, D], FP16, kind="ExternalInput").ap()
    w2_d = nc.dram_tensor("w2", [D, D], BF16, kind="ExternalInput").ap()
    b1_d = nc.dram_tensor("b1g", [P, len(GROUPS)], F32, kind="ExternalInput").ap()
    b2_d = nc.dram_tensor("b2c", [P, 1], F32, kind="ExternalInput").ap()
    outT_d = nc.dram_tensor("outT", [P, NPAD], BF16, kind="ExternalOutput").ap()

    U_g, P_g = geo["U_g"], geo["P_g"]
    OU, OF = geo["OU"], geo["OF"]

    with tile.TileContext(nc) as tc:
        with (
            tc.tile_pool(name="const", bufs=1) as cp,
            tc.tile_pool(name="u8ch", bufs=6) as u8p,
            tc.tile_pool(name="dq", bufs=6) as dqp,
            tc.tile_pool(name="f8ch", bufs=5) as f8p,
            tc.tile_pool(name="hbuf", bufs=3) as hp,
            tc.tile_pool(name="zbuf", bufs=3) as zp,
            tc.tile_pool(name="obuf", bufs=3) as op_,
            tc.tile_pool(name="ph", bufs=4, space="PSUM") as php,
            tc.tile_pool(name="pz", bufs=2, space="PSUM") as pzp,
            tc.tile_pool(name="po", bufs=2, space="PSUM") as pop,
        ):
            i16_t = cp.tile([P, P], FP16)
            nc.scalar.dma_start(i16_t[:], i16_d[:])
            i8dr_t = cp.tile([P, 2, P], FP8)
            nc.scalar.dma_start(i8dr_t[:], i8dr_d[:])
            w1_t = cp.tile([D, D], FP16)
            nc.scalar.dma_start(w1_t[:], w1_d[:])
            w2_t = cp.tile([D, D], BF16)
            nc.scalar.dma_start(w2_t[:], w2_d[:])
            b1_t = cp.tile([P, len(GROUPS)], F32)
            nc.scalar.dma_start(b1_t[:], b1_d[:])
            b2_t = cp.tile([P, 1], F32)
            nc.scalar.dma_start(b2_t[:], b2_d[:])

            PREFETCH = 3

            def issue_dmas(gi):
                w0, gw = GROUPS[gi]
                gcols = gw * P
                ug, pg = U_g[gi], P_g[gi]
                u8t = u8p.tile([P, ug, gcols], U8, tag=f"u8_{ug}_{gw}")
                nc.sync.dma_start(u8t[:], xu8_d[:, OU[gi] : OU[gi] + ug * gcols])
                f8_tiles = []
                q0 = 0
                while q0 < pg:
                    nq = min(CHF_P, pg - q0)
                    t = f8p.tile([P, nq, 2, gcols], FP8, tag=f"f8_{nq}_{gw}")
                    nc.gpsimd.dma_start(
                        t[:],
                        xf8_d[:, OF[gi] + q0 * 2 * gcols : OF[gi] + (q0 + nq) * 2 * gcols],
                    )
                    f8_tiles.append((t, q0, nq))
                    q0 += nq
                return u8t, f8_tiles

            staged = {}
            for idx in range(min(PREFETCH, len(PROC_ORDER))):
                staged[PROC_ORDER[idx]] = issue_dmas(PROC_ORDER[idx])

            def emit_w1(gk, h_sb):
                w0k, gwk = GROUPS[gk]
                gc = gwk * P
                psum_z = pzp.tile([P, GW * P], F32, tag="pz")
                nc.tensor.matmul(
                    psum_z[:, :gc], lhsT=w1_t[:], rhs=h_sb[:], start=True, stop=True
                )
                z_sb = zp.tile([P, gc], BF16, tag=f"z{gwk}")
                nc.scalar.activation(
                    z_sb[:], psum_z[:, :gc], Relu, bias=b1_t[:, gk : gk + 1]
                )
                return (gk, z_sb)

            def emit_w2(gj, z_sb):
                w0j, gwj = GROUPS[gj]
                gc = gwj * P
                psum_o = pop.tile([P, GW * P], F32, tag="po")
                nc.tensor.matmul(
                    psum_o[:, :gc], lhsT=w2_t[:], rhs=z_sb[:], start=True, stop=True
                )
                o_sb = op_.tile([P, gc], BF16, tag=f"o{gwj}")
                nc.scalar.activation(
                    o_sb[:], psum_o[:, :gc], Relu, bias=b2_t[:, :1]
                )
                nc.sync.dma_start(outT_d[:, w0j * P : (w0j + gwj) * P], o_sb[:])

            pend1 = []  # (gi, h_sb) awaiting the W1 matmul
            pend2 = []  # (gi, z_sb) awaiting the W2 matmul

            for idx, gi in enumerate(PROC_ORDER):
                w0, gw = GROUPS[gi]
                gcols = gw * P
                ug, pg = U_g[gi], P_g[gi]
                npair = ug // 2
                nsing = ug % 2
                nslot = npair + nsing
                u8t, f8_tiles = staged.pop(gi)

                dq = dqp.tile([P, nslot, gcols], FP16, tag=f"dq_{nslot}_{gw}")
                hh = (npair + 1) // 2
                for p0, p1 in ((0, hh), (hh, npair)):
                    if p1 > p0:
                        nc.vector.tensor_add(
                            dq[:, p0:p1],
                            u8t[:, p0:p1],
                            u8t[:, npair + p0 : npair + p1],
                        )
                if nsing:
                    nc.vector.tensor_copy(dq[:, npair], u8t[:, 2 * npair])

                psum_h = php.tile([P, GW * P], F32, tag="ph")
                for t, q0, nq in f8_tiles:
                    for qq in range(nq):
                        q = q0 + qq
                        nc.tensor.matmul(
                            psum_h[:, :gcols],
                            lhsT=i8dr_t[:],
                            rhs=t[:, qq],
                            start=(q == 0),
                            stop=False,
                            perf_mode=DR,
                        )
                for slot in range(nslot):
                    nc.tensor.matmul(
                        psum_h[:, :gcols],
                        lhsT=i16_t[:],
                        rhs=dq[:, slot],
                        start=(slot == 0 and pg == 0),
                        stop=(slot == nslot - 1),
                    )

                h_sb = hp.tile([P, gw * P], FP16, tag=f"h{gw}")
                nc.scalar.activation(h_sb[:], psum_h[:, :gcols], Copy)

                if pend2:
                    emit_w2(*pend2.pop(0))
                if pend1:
                    pend2.append(emit_w1(*pend1.pop(0)))
                pend1.append((gi, h_sb))

                if idx + PREFETCH < len(PROC_ORDER):
                    gnext = PROC_ORDER[idx + PREFETCH]
                    staged[gnext] = issue_dmas(gnext)

            while pend1 or pend2:
                if pend2:
                    emit_w2(*pend2.pop(0))
                if pend1:
                    pend2.append(emit_w1(*pend1.pop(0)))
    nc.compile()
    return nc


def kernel(x, edge_index, W1, b1, W2, b2, eps):
    x = np.ascontiguousarray(np.asarray(x, dtype=np.float32))
    W1 = np.asarray(W1, dtype=np.float32)
    W2 = np.asarray(W2, dtype=np.float32)
    b1 = np.asarray(b1, dtype=np.float32)
    b2 = np.asarray(b2, dtype=np.float32)
    eps_val = float(np.asarray(eps))

    node_at, geo, idx_u8, idx_f8 = _prep_host(np.asarray(edge_index))
    nc = _build_program(geo)

    codes_all = np.empty((2 * N_NODES + 1, D), dtype=np.uint8)
    codes_all[:N_NODES] = (np.clip(np.rint(x / S), -128, 127) + 128).astype(np.uint8)
    codes_all[N_NODES : 2 * N_NODES] = (
        np.clip(np.rint((1.0 + eps_val) * x / S), -128, 127) + 128
    ).astype(np.uint8)
    codes_all[2 * N_NODES] = 128

    f8_all = np.zeros((2 * N_NODES + 1, D), dtype=F8)
    f8_all[:N_NODES] = (x / S).astype(F8)

    identf = np.eye(P, dtype=np.float32).astype(np.float16)
    i8dr = np.ascontiguousarray(
        np.stack([np.eye(P, dtype=np.float32).astype(F8)] * 2, axis=1)
    )
    w1s = (S * W1).astype(np.float16)
    colsum_w1 = W1.sum(axis=0)
    b1g = np.empty((P, len(GROUPS)), dtype=np.float32)
    for gi in range(len(GROUPS)):
        b1g[:, gi] = b1 - 128.0 * S * geo["U_g"][gi] * colsum_w1
    b2c = np.ascontiguousarray(b2.reshape(P, 1))

    in_maps = []
    for c in range(N_CORES):
        xu8 = np.ascontiguousarray(codes_all[idx_u8[c]].T)
        xf8 = np.ascontiguousarray(f8_all[idx_f8[c]].T)
        in_maps.append(
            {
                "xu8": xu8,
                "xf8": xf8,
                "identf": identf,
                "ident8dr": i8dr,
                "w1s": w1s,
                "w2": W2.astype(BF),
                "b1g": b1g,
                "b2c": b2c,
            }
        )
    res = run_bass_kernel_spmd(nc, in_maps, list(range(N_CORES)))

    out = np.empty((N_NODES, D), dtype=np.float32)
    for c in range(N_CORES):
        rows = np.asarray(res.results[c]["outT"]).astype(np.float32).T
        ids = node_at[c]
        valid = ids >= 0
        out[ids[valid]] = rows[valid]
    kernel.last_results = res
    return out
